# revision 28
# baseline (speedup 1.0000x reference)
"""Multi-head causal attention with RoPE on 8 Trainium2 NeuronCores.

Sharding: 8 cores = 2 (batch) x 4 (head groups of 4 heads).  Each core
computes its batch's attention for its 4 heads and the partial output
projection over those heads; the host sums the 4 partial outputs per batch.

v4 schedule (vs v2 baseline):
  - Global interleave: the projection chains for pc>=1 and the output
    projection are woven between attention blocks, so the ACT-bound
    attention stream and the PE-bound projection stream overlap instead
    of running as separate eras.
  - Score tiles split per head: two [128,512] PSUM tiles (one bank each)
    instead of one [128,1024].  The two row-tiled S matmuls of a head
    pair drain concurrently into different banks, exp stays a contiguous
    2D read, and the PSUM budget drops to aux(3)+sp(3)+oacc(2) = 8 banks
    so all pools coexist.
  - Projection chains for pc>=1 interleave the qk and v matmuls per
    m-chunk so each LDWEIGHTS hides under the other chain's matmul.
  - sines/cosines stored bf16 (halves their DMA), output written bf16
    (halves output DMA); host accumulates partials in fp32.
"""

import numpy as np
import sys

sys.path.insert(0, "/opt/trn_rl_repo")

import concourse.bass as bass
import concourse.tile as tile
from concourse import bacc, mybir
from concourse.bass_utils import run_bass_kernel_spmd

# Problem constants (hardcoded per contract).
B = 2
S = 2048
D_MODEL = 1024
N_HEADS = 16
D_HEAD = 64
HEADS_PER_CORE = 4
N_CORES = 8

F32 = mybir.dt.float32
F32R = mybir.dt.float32r
BF16 = mybir.dt.bfloat16

QC = 512          # attention q-chunk width
N_QC = S // QC    # 4
PCW = 512         # projection column-chunk width
N_PC = S // PCW   # 4
N_KT = S // 128   # 16 k-tiles
N_MC = D_MODEL // 128  # 8 m-chunks


def r(ap):
    """View an fp32 AP as float32r for full-rate PE matmuls."""
    return ap.bitcast(F32R)


def build_nc():
    nc = bacc.Bacc(None, target_bir_lowering=False)

    xT = nc.dram_tensor("xT", [D_MODEL, S], BF16, kind="ExternalInput")
    wqkT = nc.dram_tensor("wqkT", [D_MODEL, 512], BF16, kind="ExternalInput")
    wvT = nc.dram_tensor("wvT", [D_MODEL, 256], BF16, kind="ExternalInput")
    woT = nc.dram_tensor("woT", [256, D_MODEL], BF16, kind="ExternalInput")
    cosT = nc.dram_tensor("cosT", [128, S], BF16, kind="ExternalInput")
    sinswapT = nc.dram_tensor("sinswapT", [128, S], BF16, kind="ExternalInput")
    pswap = nc.dram_tensor("pswap", [128, 128], F32R, kind="ExternalInput")
    cmask = nc.dram_tensor("cmask", [128, 256], BF16, kind="ExternalInput")
    outT = nc.dram_tensor("outT", [D_MODEL, S], BF16, kind="ExternalOutput")

    with tile.TileContext(nc) as tc:
        with (
            nc.allow_low_precision(reason="float32r/bf16 matmul operands"),
            tc.tile_pool(name="warm", bufs=1) as warm,
            tc.tile_pool(name="consts", bufs=1) as consts,
            tc.tile_pool(name="persist", bufs=1) as persist,
            tc.tile_pool(name="xt", bufs=1) as xtp,
            tc.tile_pool(name="rope", bufs=4) as rope,
            tc.tile_pool(name="pt", bufs=3) as ptp,
            tc.tile_pool(name="stg", bufs=4) as stgp,
            tc.tile_pool(name="rdp", bufs=2) as rdp,
            tc.tile_pool(name="auxP", bufs=2, space="PSUM") as auxP,
            tc.tile_pool(name="spP", bufs=2, space="PSUM") as spP,
            tc.tile_pool(name="oaccP", bufs=1, space="PSUM") as oaccP,
        ):
            # ---- preload the exp table set (~2.7us) while DMAs stream,
            # so the first real exp doesn't pay it ----
            wtile = warm.tile([1, 16], F32, tag="warm")
            wout = warm.tile([1, 16], F32, tag="warmo")
            nc.vector.memset(wtile[:], 0.0)
            nc.scalar.activation(wout[:], wtile[:],
                                 mybir.ActivationFunctionType.Exp)

            # ---- constant + input tiles; DMA emission order matters ----
            pswap_t = consts.tile([128, 128], F32R, tag="pswap")
            cmask_t = consts.tile([128, 2, 128], BF16, tag="cmask")

            cos_t = xtp.tile([128, S], BF16, tag="cos")
            sin_t = xtp.tile([128, S], BF16, tag="sin")
            xt = [xtp.tile([128, S], BF16, tag=f"x{mc}", name=f"xt{mc}")
                  for mc in range(N_MC)]

            # wv first, then the first half of x in full 1024-col slices
            # (2KB DMA lines keep the stream at full rate) so the first v
            # chains and the pc1 filler units are fed in time
            wv_t = []
            for mc in range(N_MC):
                w2 = consts.tile([128, 256], BF16, tag=f"wv{mc}", name=f"wv{mc}")
                nc.sync.dma_start(w2[:], wvT[mc * 128:(mc + 1) * 128, :])
                wv_t.append(w2)
            for mc in range(N_MC):
                nc.sync.dma_start(xt[mc][:, 0:1024],
                                  xT[mc * 128:(mc + 1) * 128, 0:1024])
            nc.sync.dma_start(sin_t[:, 0:512], sinswapT[:, 0:512])
            nc.sync.dma_start(cos_t[:, 0:512], cosT[:, 0:512])
            wqk_t = []
            for mc in range(N_MC):
                w1 = consts.tile([128, 512], BF16, tag=f"wqk{mc}", name=f"wqk{mc}")
                nc.sync.dma_start(w1[:], wqkT[mc * 128:(mc + 1) * 128, :])
                wqk_t.append(w1)
            nc.sync.dma_start(sin_t[:, 512:1024], sinswapT[:, 512:1024])
            nc.sync.dma_start(cos_t[:, 512:1024], cosT[:, 512:1024])
            nc.sync.dma_start(pswap_t[:], pswap[:])
            nc.sync.dma_start(cmask_t[:], cmask.rearrange("p (h q) -> p h q", h=2))
            for mc in range(N_MC):
                nc.sync.dma_start(xt[mc][:, 1024:2048],
                                  xT[mc * 128:(mc + 1) * 128, 1024:2048])
            wo_t = []
            for t in range(2):
                w3 = consts.tile([128, D_MODEL], BF16, tag=f"wo{t}", name=f"wo{t}")
                nc.sync.dma_start(w3[:], woT[t * 128:(t + 1) * 128, :])
                wo_t.append(w3)
            for csl in (slice(1024, 1536), slice(1536, 2048)):
                nc.sync.dma_start(sin_t[:, csl], sinswapT[:, csl])
                nc.sync.dma_start(cos_t[:, csl], cosT[:, csl])

            # ---- persistent intermediates ----
            qk_t = [persist.tile([128, S], BF16, tag=f"qk{i}", name=f"qk{i}")
                    for i in range(4)]
            v_t = [persist.tile([128, 4, 65], BF16, tag=f"v{kt}", name=f"v{kt}")
                   for kt in range(N_KT)]
            o_t = [persist.tile([128, S], BF16, tag=f"o{t}", name=f"o{t}")
                   for t in range(2)]
            for kt in range(N_KT):
                nc.vector.memset(v_t[kt][:, :, 64:65], 1.0)

            # ======== projection pieces ========
            def v_chain(kt):
                vp = auxP.tile([128, 256], F32, tag="aux", name=f"vp{kt}")
                for mc in range(N_MC):
                    nc.tensor.matmul(
                        vp[:],
                        xt[mc][:, kt * 128:(kt + 1) * 128],
                        wv_t[mc][:],
                        start=(mc == 0), stop=(mc == N_MC - 1),
                    )
                nc.scalar.copy(
                    v_t[kt][:, :, 0:64], vp.rearrange("p (h d) -> p h d", h=4))

            # Deferred RoPE rotation: the pair-swap matmul of chunk i is
            # emitted just before chunk i+1's aux allocations and writes
            # INTO chunk i's ps tile (the chunk is done with it), so no
            # third aux buffer is needed.  Invariant: pop_rope() must run
            # before any subsequent auxP allocation (else the rot write
            # would land in a reused buffer).
            rope_pend = {}

            def rope_tail(pc, mt, ps):
                csl = slice(pc * PCW, (pc + 1) * PCW)
                u = rope.tile([128, PCW], F32R, tag="u")
                nc.vector.tensor_mul(u[:], ps[:], sin_t[:, csl])
                t1 = rope.tile([128, PCW], F32, tag="t1")
                nc.vector.tensor_mul(t1[:], ps[:], cos_t[:, csl])
                rope_pend["p"] = (pc, mt, u, t1, ps)

            def pop_rope():
                if "p" in rope_pend:
                    ppc, pmt, pu, pt1, pps = rope_pend.pop("p")
                    pcsl = slice(ppc * PCW, (ppc + 1) * PCW)
                    nc.tensor.matmul(pps[:], r(pswap_t[:]), r(pu[:]),
                                     start=True, stop=True)
                    nc.vector.tensor_add(qk_t[pmt][:, pcsl], pt1[:], pps[:])

            def qk_chain(pc, mt):
                pop_rope()
                csl = slice(pc * PCW, (pc + 1) * PCW)
                ps = auxP.tile([128, PCW], F32, tag="aux", name=f"ps{pc}_{mt}")
                for mc in range(N_MC):
                    nc.tensor.matmul(
                        ps[:],
                        wqk_t[mc][:, mt * 128:(mt + 1) * 128],
                        xt[mc][:, csl],
                        start=(mc == 0), stop=(mc == N_MC - 1),
                    )
                rope_tail(pc, mt, ps)

            def make_unit_halves(pc, mt, kt):
                """Combined qk(mt) + v(kt) chains.  half1 pops the
                pending rotation, allocates, and runs the first v matmuls;
                half2 interleaves the qk chain with the remaining v matmuls
                so each LDWEIGHTS hides under the other chain's matmul.
                Halves must be emitted at adjacent slots: nothing else may
                allocate aux tiles while the chains are half-open."""
                u = mt
                csl = slice(pc * PCW, (pc + 1) * PCW)
                st = {}

                def v_mm(vp, mc):
                    nc.tensor.matmul(
                        vp[:],
                        xt[mc][:, kt * 128:(kt + 1) * 128],
                        wv_t[mc][:],
                        start=(mc == 0), stop=(mc == N_MC - 1),
                    )

                def half1():
                    pop_rope()
                    st["vp"] = auxP.tile([128, 256], F32, tag="aux",
                                         name=f"vp{kt}")
                    st["ps"] = auxP.tile([128, PCW], F32, tag="aux",
                                         name=f"ps{pc}_{u}")
                    for mc in range(4):
                        v_mm(st["vp"], mc)

                def half2():
                    ps, vp = st["ps"], st["vp"]
                    for mc in range(N_MC):
                        nc.tensor.matmul(
                            ps[:],
                            wqk_t[mc][:, u * 128:(u + 1) * 128],
                            xt[mc][:, csl],
                            start=(mc == 0), stop=(mc == N_MC - 1),
                        )
                        if mc >= 4:
                            v_mm(vp, mc)
                    nc.vector.tensor_copy(
                        v_t[kt][:, :, 0:64],
                        vp.rearrange("p (h d) -> p h d", h=4))
                    rope_tail(pc, u, ps)

                return half1, half2

            # ======== attention pieces ========
            def emit_S(b):
                t, qc, kt = b
                j = kt - qc * (QC // 128)
                off = max(0, j) * 128
                diag = j >= 0
                k_tile = qk_t[2 + t]
                q_tile = qk_t[t]
                sp = spP.tile([128, 2 * QC], F32, tag="scores",
                              name=f"sc{t}_{qc}_{kt}")
                for h in range(2):
                    hsl = slice(h * 64, (h + 1) * 64)
                    nc.tensor.matmul(
                        sp[:, h * QC + off:(h + 1) * QC],
                        k_tile[hsl, kt * 128:(kt + 1) * 128],
                        q_tile[hsl, qc * QC + off:(qc + 1) * QC],
                        start=True, stop=True,
                    )
                return sp, off, diag

            def emit_exp(sp, off, diag):
                pt = ptp.tile([128, 2 * QC], BF16, tag="pt")
                sp2 = sp.rearrange("p (h q) -> p h q", h=2)
                pt2 = pt.rearrange("p (h q) -> p h q", h=2)
                nc.scalar.activation(
                    pt2[:, :, off:QC], sp2[:, :, off:QC],
                    mybir.ActivationFunctionType.Exp,
                )
                if diag:
                    # zero the strictly-above-diagonal pattern entries of
                    # the boundary 128-col sub-block (GpSimd; off the PE
                    # and off the exp critical path — kt runs descending
                    # so later blocks hide this latency)
                    nc.gpsimd.tensor_mul(
                        pt2[:, :, off:off + 128],
                        pt2[:, :, off:off + 128],
                        cmask_t[:],
                    )
                return pt

            def emit_PV(b, oacc, pt, off):
                t, qc, kt = b
                nkt = (qc + 1) * (QC // 128)
                # kt runs DESCENDING: the first processed (kt==nkt-1) opens
                # the accumulation group (its start clears the whole bank;
                # columns it doesn't cover are filled by later kts via the
                # has_written overwrite path), kt==0 closes it
                for h in range(2):
                    nc.tensor.matmul(
                        oacc[:, h * QC + off:(h + 1) * QC],
                        v_t[kt][:, 2 * t + h, :],
                        pt[:, h * QC + off:(h + 1) * QC],
                        start=(kt == nkt - 1), stop=(kt == 0),
                    )

            def norm_pair(t, qc, oacc, last=False):
                qsl = slice(qc * QC, (qc + 1) * QC)
                rdr = rdp.tile([1, 2 * QC], F32, tag="rdr", bufs=1)
                if last:
                    # final pair: ACT is idle now; skip the staging copy and
                    # normalize straight out of PSUM to shorten the tail
                    stg = oacc
                    nc.scalar.copy(rdr[:], oacc[64:65, :])
                else:
                    stg = stgp.tile([65, 2 * QC], F32, tag="att", bufs=2,
                                    name=f"stg{t}_{qc}")
                    # split the staging copy across DVE+ACT so the oacc
                    # bank is released ~2x sooner for the next pair
                    nc.vector.tensor_copy(stg[:, 0:QC], oacc[:, 0:QC])
                    nc.scalar.copy(stg[:, QC:2 * QC], oacc[:, QC:2 * QC])
                    # denominator row to partition 0: the custom-DVE recip
                    # mis-reads partition-offset inputs
                    nc.vector.tensor_copy(rdr[:], stg[64:65, :])
                rd = rdp.tile([1, 2 * QC], F32, tag="rd")
                nc.vector.reciprocal_approx_fast(rd[:, 0:QC], rdr[:, 0:QC])
                nc.vector.reciprocal_approx_fast(rd[:, QC:2 * QC],
                                                 rdr[:, QC:2 * QC])
                bc = rdp.tile([64, 2 * QC], F32, tag="bc")
                nc.gpsimd.partition_broadcast(bc[:, 0:QC], rd[:, 0:QC])
                nc.gpsimd.partition_broadcast(bc[:, QC:2 * QC], rd[:, QC:2 * QC])
                for h in range(2):
                    nc.vector.tensor_mul(
                        o_t[t][h * 64:(h + 1) * 64, qsl],
                        stg[0:64, h * QC:(h + 1) * QC],
                        bc[:, h * QC:(h + 1) * QC])

            def op_chunk(qc, mt, tail=False):
                def emit():
                    qsl = slice(qc * QC, (qc + 1) * QC)
                    op = auxP.tile([128, QC], F32, tag="aux",
                                   name=f"op{qc}_{mt}")
                    for t in range(2):
                        nc.tensor.matmul(
                            op[:],
                            wo_t[t][:, mt * 128:(mt + 1) * 128],
                            o_t[t][:, qsl],
                            start=(t == 0), stop=(t == 1),
                        )
                    st = stgp.tile([128, QC], BF16, tag="st")
                    # alternate engines so the staging copies split across
                    # ACT and DVE (both have slack vs the PE)
                    if mt % 2 == 0:
                        nc.scalar.copy(st[:], op[:])
                    else:
                        nc.vector.tensor_copy(st[:], op[:])
                    nc.sync.dma_start(
                        outT[mt * 128:(mt + 1) * 128, qsl], st[:])
                return emit

            # ======== era 0: pc0 projections (DMA-paced) ========
            # v chains woven between qk chains (the v matmuls cover the
            # rope mul->rot->add latency of the previous qk chain); mt
            # order 0,2 first so the t=0 head pair's q AND k rotations are
            # flushed before the first attention block reads them
            v_chain(0)
            v_chain(1)
            qk_chain(0, 0)
            v_chain(2)
            qk_chain(0, 2)
            v_chain(3)
            pop_rope()  # rot(mt2): t=0 attention is now unblocked

            # ======== global interleaved stream ========
            # kt runs DESCENDING within each (t, qc) so the diagonal blocks
            # (whose pattern needs the GpSimd mask) come first and their
            # mask latency hides under the following below-diagonal blocks
            blocks = [(t, qc, kt)
                      for qc in range(N_QC)
                      for t in range(2)
                      for kt in reversed(range((qc + 1) * (QC // 128)))]

            # proj fillers: pc0's remaining head-pair-1 chains first, then
            # unit halves of pc1-3 at adjacent slot pairs (keeps other aux
            # allocations out of half-open chains).  qk chains run in mt
            # order 0,2,1,3 (t=0's q/k rotations first); v chains run kt
            # DESCENDING (matching the attention order), so units pair
            # mt_order[j] with kt=4*pc+3-j.
            filler_map = {
                0: lambda: qk_chain(0, 1),
                1: lambda: qk_chain(0, 3),
                2: pop_rope,
            }
            base = {1: 3, 2: 16, 3: 32}
            MT_ORDER = (0, 2, 1, 3)
            for pc in range(1, N_PC):
                for j in range(4):
                    h1, h2 = make_unit_halves(pc, MT_ORDER[j], 4 * pc + 3 - j)
                    filler_map[base[pc] + 2 * j] = h1
                    filler_map[base[pc] + 2 * j + 1] = h2
                filler_map[base[pc] + 8] = pop_rope

            op_pending = []
            oacc_cur = None
            pend_S = None
            for i, b in enumerate(blocks):
                t, qc, kt = b
                nkt = (qc + 1) * (QC // 128)
                if kt == nkt - 1:
                    oacc_cur = oaccP.tile([65, 2 * QC], F32, tag="oacc",
                                          name=f"oacc{t}_{qc}")
                    # op chunks for qc-1 become safe to pop one pair
                    # after their norms were emitted
                    if t == 1 and qc >= 1:
                        for mt in range(N_MC):
                            op_pending.append(op_chunk(qc - 1, mt))
                if pend_S is None:
                    pend_S = emit_S(b)
                sp, off, diag = pend_S
                f = filler_map.get(i)
                if f is not None:
                    f()
                elif i >= 10 and op_pending:
                    op_pending.pop(0)()
                pend_S = emit_S(blocks[i + 1]) \
                    if i + 1 < len(blocks) else None
                pt = emit_exp(sp, off, diag)
                emit_PV(b, oacc_cur, pt, off)
                if kt == 0:
                    norm_pair(t, qc, oacc_cur, last=(i == len(blocks) - 1))
            # tail: remaining queued chunks + outproj of the last q-chunk
            while op_pending:
                op_pending.pop(0)()
            for mt in range(N_MC):
                op_chunk(N_QC - 1, mt, tail=True)()

    nc.compile()
    return nc


def make_in_maps(x, key_weight, query_weight, value_weight, output_weight,
                 sines, cosines):
    """Host-side sharding + layout prep. Returns list of 8 per-core dicts."""
    import ml_dtypes
    bf16 = ml_dtypes.bfloat16
    f32 = np.float32

    # RoPE factor tiles [128, S]: row r (within a 64-channel head block)
    # carries cos/sin of pair index (r % 64) // 2; sin rows get sign -1 on
    # even rows (out_even = e*c - o*s) and +1 on odd rows.
    idx = np.tile(np.repeat(np.arange(D_HEAD // 2), 2), 2)  # [128]
    sign = np.tile(np.array([-1.0, 1.0], dtype=f32), 64)
    cosT = np.ascontiguousarray(cosines.T[idx, :]).astype(bf16)          # [128, S]
    sinT = sines.T[idx, :] * sign[:, None]
    # rows pre-permuted by the pair swap so that P @ (x * sinswapT) equals
    # rot(x) * sinT
    rr128 = np.arange(128) ^ 1
    sinswapT = np.ascontiguousarray(sinT[rr128, :]).astype(bf16)

    psw = np.zeros((128, 128), dtype=f32)
    rr = np.arange(128)
    psw[rr, rr ^ 1] = 1.0

    # post-exp causal zeroing: keep k <= q within the boundary sub-block;
    # duplicated side by side so one GpSimd op masks both heads
    cm = (np.arange(128)[:, None] <= np.arange(128)[None, :]).astype(np.float32)
    cmask = np.ascontiguousarray(np.concatenate([cm, cm], axis=1)).astype(bf16)

    in_maps = []
    for c in range(N_CORES):
        b, g = divmod(c, 4)
        hs = slice(g * HEADS_PER_CORE, (g + 1) * HEADS_PER_CORE)
        xTb = np.ascontiguousarray(x[b].T).astype(bf16)
        wqT = np.ascontiguousarray(
            query_weight[hs].transpose(2, 0, 1).reshape(D_MODEL, 256)).astype(bf16)
        wkT = np.ascontiguousarray(
            key_weight[hs].transpose(2, 0, 1).reshape(D_MODEL, 256)).astype(bf16)
        wvT = np.ascontiguousarray(
            value_weight[hs].transpose(2, 0, 1).reshape(D_MODEL, 256)).astype(bf16)
        woT = np.ascontiguousarray(
            output_weight[:, hs, :].transpose(1, 2, 0).reshape(256, D_MODEL)
        ).astype(bf16)
        in_maps.append({
            "xT": xTb,
            "wqkT": np.concatenate([wqT, wkT], axis=1),
            "wvT": wvT,
            "woT": woT,
            "cosT": cosT,
            "sinswapT": sinswapT,
            "pswap": psw,
            "cmask": cmask,
        })
    return in_maps


_NC_CACHE = None


def get_nc():
    global _NC_CACHE
    if _NC_CACHE is None:
        _NC_CACHE = build_nc()
    return _NC_CACHE


def kernel(x, key_weight, query_weight, value_weight, output_weight,
           sines, cosines, _trace=False, _trace_kwargs=None):
    in_maps = make_in_maps(x, key_weight, query_weight, value_weight,
                           output_weight, sines, cosines)
    nc = get_nc()
    kw = {}
    if _trace:
        kw = dict(trace=True, **(_trace_kwargs or {}))
    res = run_bass_kernel_spmd(nc, in_maps, core_ids=list(range(N_CORES)), **kw)
    out = np.zeros((B, S, D_MODEL), dtype=np.float32)
    for c in range(N_CORES):
        b = c // 4
        out[b] += res.results[c]["outT"].astype(np.float32).T
    kernel.last_result = res
    return out


# revision 38
# speedup vs baseline: 1.8088x; 1.8088x over previous
"""Multi-head causal attention with RoPE on 8 Trainium2 NeuronCores.

Sharding: 8 cores = 2 (batch) x 4 (head groups of 4 heads).  Each core
computes its batch's attention for its 4 heads and the partial output
projection over those heads; the host sums the 4 partial outputs per batch.

v4 schedule (vs v2 baseline):
  - Global interleave: the projection chains for pc>=1 and the output
    projection are woven between attention blocks, so the ACT-bound
    attention stream and the PE-bound projection stream overlap instead
    of running as separate eras.
  - Score tiles split per head: two [128,512] PSUM tiles (one bank each)
    instead of one [128,1024].  The two row-tiled S matmuls of a head
    pair drain concurrently into different banks, exp stays a contiguous
    2D read, and the PSUM budget drops to aux(3)+sp(3)+oacc(2) = 8 banks
    so all pools coexist.
  - Projection chains for pc>=1 interleave the qk and v matmuls per
    m-chunk so each LDWEIGHTS hides under the other chain's matmul.
  - sines/cosines stored bf16 (halves their DMA), output written bf16
    (halves output DMA); host accumulates partials in fp32.
"""

import numpy as np
import sys

sys.path.insert(0, "/opt/trn_rl_repo")

import concourse.bass as bass
import concourse.tile as tile
from concourse import bacc, mybir
from concourse.bass_utils import run_bass_kernel_spmd

# Problem constants (hardcoded per contract).
B = 2
S = 2048
D_MODEL = 1024
N_HEADS = 16
D_HEAD = 64
HEADS_PER_CORE = 4
N_CORES = 8

F32 = mybir.dt.float32
F32R = mybir.dt.float32r
BF16 = mybir.dt.bfloat16

QC = 512          # attention q-chunk width
N_QC = S // QC    # 4
PCW = 512         # projection column-chunk width
N_PC = S // PCW   # 4
N_KT = S // 128   # 16 k-tiles
N_MC = D_MODEL // 128  # 8 m-chunks


def r(ap):
    """View an fp32 AP as float32r for full-rate PE matmuls."""
    return ap.bitcast(F32R)


def build_nc():
    nc = bacc.Bacc(None, target_bir_lowering=False)

    xT = nc.dram_tensor("xT", [D_MODEL, S], BF16, kind="ExternalInput")
    wqkT = nc.dram_tensor("wqkT", [D_MODEL, 512], BF16, kind="ExternalInput")
    wvT = nc.dram_tensor("wvT", [D_MODEL, 256], BF16, kind="ExternalInput")
    woT = nc.dram_tensor("woT", [256, D_MODEL], BF16, kind="ExternalInput")
    cosT = nc.dram_tensor("cosT", [128, S], BF16, kind="ExternalInput")
    sinswapT = nc.dram_tensor("sinswapT", [128, S], BF16, kind="ExternalInput")
    pswap = nc.dram_tensor("pswap", [128, 128], F32R, kind="ExternalInput")
    triA = nc.dram_tensor("triA", [128, 128], BF16, kind="ExternalInput")
    identB = nc.dram_tensor("identB", [128, 128], BF16, kind="ExternalInput")
    outT = nc.dram_tensor("outT", [D_MODEL, S], BF16, kind="ExternalOutput")

    with tile.TileContext(nc) as tc:
        with (
            nc.allow_low_precision(reason="float32r/bf16 matmul operands"),
            tc.tile_pool(name="warm", bufs=1) as warm,
            tc.tile_pool(name="consts", bufs=1) as consts,
            tc.tile_pool(name="persist", bufs=1) as persist,
            tc.tile_pool(name="xt", bufs=1) as xtp,
            tc.tile_pool(name="rope", bufs=4) as rope,
            tc.tile_pool(name="pt", bufs=3) as ptp,
            tc.tile_pool(name="stg", bufs=4) as stgp,
            tc.tile_pool(name="rdp", bufs=2) as rdp,
            tc.tile_pool(name="auxP", bufs=2, space="PSUM") as auxP,
            tc.tile_pool(name="spP", bufs=2, space="PSUM") as spP,
            tc.tile_pool(name="oaccP", bufs=1, space="PSUM") as oaccP,
        ):
            # ---- preload the exp table set (~2.7us) while DMAs stream,
            # so the first real exp doesn't pay it ----
            wtile = warm.tile([1, 16], F32, tag="warm")
            wout = warm.tile([1, 16], F32, tag="warmo")
            nc.vector.memset(wtile[:], 0.0)
            nc.scalar.activation(wout[:], wtile[:],
                                 mybir.ActivationFunctionType.Exp)

            # ---- constant + input tiles; DMA emission order matters ----
            pswap_t = consts.tile([128, 128], F32R, tag="pswap")
            triA_t = consts.tile([128, 128], BF16, tag="triA")
            identB_t = consts.tile([128, 128], BF16, tag="identB")

            cos_t = xtp.tile([128, S], BF16, tag="cos")
            sin_t = xtp.tile([128, S], BF16, tag="sin")
            xt = [xtp.tile([128, S], BF16, tag=f"x{mc}", name=f"xt{mc}")
                  for mc in range(N_MC)]

            # wv first, then the first half of x in full 1024-col slices
            # (2KB DMA lines keep the stream at full rate) so the first v
            # chains and the pc1 filler units are fed in time
            wv_t = []
            for mc in range(N_MC):
                w2 = consts.tile([128, 256], BF16, tag=f"wv{mc}", name=f"wv{mc}")
                nc.sync.dma_start(w2[:], wvT[mc * 128:(mc + 1) * 128, :])
                wv_t.append(w2)
            for mc in range(N_MC):
                nc.sync.dma_start(xt[mc][:, 0:512],
                                  xT[mc * 128:(mc + 1) * 128, 0:512])
            nc.sync.dma_start(sin_t[:, 0:512], sinswapT[:, 0:512])
            nc.sync.dma_start(cos_t[:, 0:512], cosT[:, 0:512])
            wqk_t = []
            for mc in range(N_MC):
                w1 = consts.tile([128, 512], BF16, tag=f"wqk{mc}", name=f"wqk{mc}")
                nc.sync.dma_start(w1[:], wqkT[mc * 128:(mc + 1) * 128, :])
                wqk_t.append(w1)
            for mc in range(N_MC):
                nc.sync.dma_start(xt[mc][:, 512:1024],
                                  xT[mc * 128:(mc + 1) * 128, 512:1024])
            nc.sync.dma_start(sin_t[:, 512:1024], sinswapT[:, 512:1024])
            nc.sync.dma_start(cos_t[:, 512:1024], cosT[:, 512:1024])
            nc.sync.dma_start(pswap_t[:], pswap[:])
            nc.sync.dma_start(triA_t[:], triA[:])
            nc.sync.dma_start(identB_t[:], identB[:])
            for mc in range(N_MC):
                nc.sync.dma_start(xt[mc][:, 1024:2048],
                                  xT[mc * 128:(mc + 1) * 128, 1024:2048])
            wo_t = []
            for t in range(2):
                w3 = consts.tile([128, D_MODEL], BF16, tag=f"wo{t}", name=f"wo{t}")
                nc.sync.dma_start(w3[:], woT[t * 128:(t + 1) * 128, :])
                wo_t.append(w3)
            for csl in (slice(1024, 1536), slice(1536, 2048)):
                nc.sync.dma_start(sin_t[:, csl], sinswapT[:, csl])
                nc.sync.dma_start(cos_t[:, csl], cosT[:, csl])

            # ---- persistent intermediates ----
            qk_t = [persist.tile([128, S], BF16, tag=f"qk{i}", name=f"qk{i}")
                    for i in range(4)]
            v_t = [persist.tile([128, 4, 65], BF16, tag=f"v{kt}", name=f"v{kt}")
                   for kt in range(N_KT)]
            o_t = [persist.tile([128, S], BF16, tag=f"o{t}", name=f"o{t}")
                   for t in range(2)]
            for kt in range(N_KT):
                nc.vector.memset(v_t[kt][:, :, 64:65], 1.0)

            # ======== projection pieces ========
            def v_chain(kt):
                vp = auxP.tile([128, 256], F32, tag="aux", name=f"vp{kt}")
                for mc in range(N_MC):
                    nc.tensor.matmul(
                        vp[:],
                        xt[mc][:, kt * 128:(kt + 1) * 128],
                        wv_t[mc][:],
                        start=(mc == 0), stop=(mc == N_MC - 1),
                    )
                nc.scalar.copy(
                    v_t[kt][:, :, 0:64], vp.rearrange("p (h d) -> p h d", h=4))

            # Deferred RoPE rotation: the pair-swap matmul of chunk i is
            # emitted just before chunk i+1's aux allocations and writes
            # INTO chunk i's ps tile (the chunk is done with it), so no
            # third aux buffer is needed.  Invariant: pop_rope() must run
            # before any subsequent auxP allocation (else the rot write
            # would land in a reused buffer).
            rope_pend = {}

            def rope_tail(pc, mt, ps):
                csl = slice(pc * PCW, (pc + 1) * PCW)
                u = rope.tile([128, PCW], F32R, tag="u")
                nc.vector.tensor_mul(u[:], ps[:], sin_t[:, csl])
                t1 = rope.tile([128, PCW], F32, tag="t1")
                nc.vector.tensor_mul(t1[:], ps[:], cos_t[:, csl])
                rope_pend["p"] = (pc, mt, u, t1, ps)

            def pop_rope():
                if "p" in rope_pend:
                    ppc, pmt, pu, pt1, pps = rope_pend.pop("p")
                    pcsl = slice(ppc * PCW, (ppc + 1) * PCW)
                    nc.tensor.matmul(pps[:], r(pswap_t[:]), r(pu[:]),
                                     start=True, stop=True)
                    nc.vector.tensor_add(qk_t[pmt][:, pcsl], pt1[:], pps[:])

            def qk_chain(pc, mt):
                pop_rope()
                csl = slice(pc * PCW, (pc + 1) * PCW)
                ps = auxP.tile([128, PCW], F32, tag="aux", name=f"ps{pc}_{mt}")
                for mc in range(N_MC):
                    nc.tensor.matmul(
                        ps[:],
                        wqk_t[mc][:, mt * 128:(mt + 1) * 128],
                        xt[mc][:, csl],
                        start=(mc == 0), stop=(mc == N_MC - 1),
                    )
                rope_tail(pc, mt, ps)

            def make_unit_halves(pc, mt, kt):
                """Combined qk(mt) + v(kt) chains.  half1 pops the
                pending rotation, allocates, and runs the first v matmuls;
                half2 interleaves the qk chain with the remaining v matmuls
                so each LDWEIGHTS hides under the other chain's matmul.
                Halves must be emitted at adjacent slots: nothing else may
                allocate aux tiles while the chains are half-open."""
                u = mt
                csl = slice(pc * PCW, (pc + 1) * PCW)
                st = {}

                def v_mm(vp, mc):
                    nc.tensor.matmul(
                        vp[:],
                        xt[mc][:, kt * 128:(kt + 1) * 128],
                        wv_t[mc][:],
                        start=(mc == 0), stop=(mc == N_MC - 1),
                    )

                def half1():
                    pop_rope()
                    st["vp"] = auxP.tile([128, 256], F32, tag="aux",
                                         name=f"vp{kt}")
                    st["ps"] = auxP.tile([128, PCW], F32, tag="aux",
                                         name=f"ps{pc}_{u}")
                    for mc in range(4):
                        v_mm(st["vp"], mc)

                def half2():
                    ps, vp = st["ps"], st["vp"]
                    for mc in range(N_MC):
                        nc.tensor.matmul(
                            ps[:],
                            wqk_t[mc][:, u * 128:(u + 1) * 128],
                            xt[mc][:, csl],
                            start=(mc == 0), stop=(mc == N_MC - 1),
                        )
                        if mc >= 4:
                            v_mm(vp, mc)
                    nc.vector.tensor_copy(
                        v_t[kt][:, :, 0:64],
                        vp.rearrange("p (h d) -> p h d", h=4))
                    rope_tail(pc, u, ps)

                return half1, half2

            # ======== attention pieces ========
            def emit_S(b):
                t, qc, kt = b
                j = kt - qc * (QC // 128)
                off = max(0, j) * 128
                diag = j >= 0
                k_tile = qk_t[2 + t]
                q_tile = qk_t[t]
                sp = spP.tile([128, 2 * QC], F32, tag="scores",
                              name=f"sc{t}_{qc}_{kt}")
                for h in range(2):
                    hsl = slice(h * 64, (h + 1) * 64)
                    nc.tensor.matmul(
                        sp[:, h * QC + off:(h + 1) * QC],
                        k_tile[hsl, kt * 128:(kt + 1) * 128],
                        q_tile[hsl, qc * QC + off:(qc + 1) * QC],
                        start=True, stop=not diag,
                    )
                if diag:
                    # add -1e9 above the causal diagonal (triA.T @ I)
                    for h in range(2):
                        nc.tensor.matmul(
                            sp[:, h * QC + off: h * QC + off + 128],
                            triA_t[:], identB_t[:],
                            start=False, stop=True,
                        )
                return sp, off, diag

            def emit_exp(sp, off, diag):
                pt = ptp.tile([128, 2 * QC], BF16, tag="pt")
                sp2 = sp.rearrange("p (h q) -> p h q", h=2)
                pt2 = pt.rearrange("p (h q) -> p h q", h=2)
                nc.scalar.activation(
                    pt2[:, :, off:QC], sp2[:, :, off:QC],
                    mybir.ActivationFunctionType.Exp,
                )
                return pt

            def emit_PV(b, oacc, pt, off):
                t, qc, kt = b
                nkt = (qc + 1) * (QC // 128)
                for h in range(2):
                    nc.tensor.matmul(
                        oacc[:, h * QC + off:(h + 1) * QC],
                        v_t[kt][:, 2 * t + h, :],
                        pt[:, h * QC + off:(h + 1) * QC],
                        start=(kt == 0), stop=(kt == nkt - 1),
                    )

            def norm_pair(t, qc, oacc, last=False):
                qsl = slice(qc * QC, (qc + 1) * QC)
                rdr = rdp.tile([1, 2 * QC], F32, tag="rdr", bufs=1)
                if last:
                    # final pair: ACT is idle now; skip the staging copy and
                    # normalize straight out of PSUM to shorten the tail
                    stg = oacc
                    nc.scalar.copy(rdr[:], oacc[64:65, :])
                else:
                    stg = stgp.tile([65, 2 * QC], F32, tag="att", bufs=2,
                                    name=f"stg{t}_{qc}")
                    # split the staging copy across DVE+ACT so the oacc
                    # bank is released ~2x sooner for the next pair
                    nc.vector.tensor_copy(stg[:, 0:QC], oacc[:, 0:QC])
                    nc.scalar.copy(stg[:, QC:2 * QC], oacc[:, QC:2 * QC])
                    # denominator row to partition 0: the custom-DVE recip
                    # mis-reads partition-offset inputs
                    nc.vector.tensor_copy(rdr[:], stg[64:65, :])
                rd = rdp.tile([1, 2 * QC], F32, tag="rd")
                nc.vector.reciprocal_approx_fast(rd[:, 0:QC], rdr[:, 0:QC])
                nc.vector.reciprocal_approx_fast(rd[:, QC:2 * QC],
                                                 rdr[:, QC:2 * QC])
                bc = rdp.tile([64, 2 * QC], F32, tag="bc")
                nc.gpsimd.partition_broadcast(bc[:, 0:QC], rd[:, 0:QC])
                nc.gpsimd.partition_broadcast(bc[:, QC:2 * QC], rd[:, QC:2 * QC])
                for h in range(2):
                    nc.vector.tensor_mul(
                        o_t[t][h * 64:(h + 1) * 64, qsl],
                        stg[0:64, h * QC:(h + 1) * QC],
                        bc[:, h * QC:(h + 1) * QC])

            def op_chunk(qc, mt, tail=False):
                def emit():
                    qsl = slice(qc * QC, (qc + 1) * QC)
                    op = auxP.tile([128, QC], F32, tag="aux",
                                   name=f"op{qc}_{mt}")
                    for t in range(2):
                        nc.tensor.matmul(
                            op[:],
                            wo_t[t][:, mt * 128:(mt + 1) * 128],
                            o_t[t][:, qsl],
                            start=(t == 0), stop=(t == 1),
                        )
                    st = stgp.tile([128, QC], BF16, tag="st")
                    # in the drain tail ACT is idle: alternate engines so the
                    # staging copies pipeline 2x
                    if tail and mt % 2 == 0:
                        nc.scalar.copy(st[:], op[:])
                    else:
                        nc.vector.tensor_copy(st[:], op[:])
                    nc.sync.dma_start(
                        outT[mt * 128:(mt + 1) * 128, qsl], st[:])
                return emit

            # ======== era 0: pc0 projections (DMA-paced) ========
            # v chains woven between qk chains (the v matmuls cover the
            # rope mul->rot->add latency of the previous qk chain); mt
            # order 0,2 first so the t=0 head pair's q AND k rotations are
            # flushed before the first attention block reads them
            v_chain(0)
            v_chain(1)
            qk_chain(0, 0)
            v_chain(2)
            qk_chain(0, 2)
            v_chain(3)
            pop_rope()  # rot(mt2): t=0 attention is now unblocked

            # ======== global interleaved stream ========
            blocks = [(t, qc, kt)
                      for qc in range(N_QC)
                      for t in range(2)
                      for kt in range((qc + 1) * (QC // 128))]

            # proj fillers: pc0's remaining head-pair-1 chains first, then
            # unit halves of pc1-3 at adjacent slot pairs (keeps other aux
            # allocations out of half-open chains), placed so every
            # rotation lands before its first attention reader
            filler_map = {
                0: lambda: qk_chain(0, 1),
                1: lambda: qk_chain(0, 3),
                2: pop_rope,
            }
            base = {1: 3, 2: 16, 3: 32}
            for pc in range(1, N_PC):
                for u in range(4):
                    h1, h2 = make_unit_halves(pc, u, 4 * pc + u)
                    filler_map[base[pc] + 2 * u] = h1
                    filler_map[base[pc] + 2 * u + 1] = h2
                filler_map[base[pc] + 8] = pop_rope

            op_pending = []
            oacc_cur = None
            pend_S = None
            for i, b in enumerate(blocks):
                t, qc, kt = b
                nkt = (qc + 1) * (QC // 128)
                if kt == 0:
                    oacc_cur = oaccP.tile([65, 2 * QC], F32, tag="oacc",
                                          name=f"oacc{t}_{qc}")
                    # op chunks for qc-1 become safe to pop one pair
                    # after their norms were emitted
                    if t == 1 and qc >= 1:
                        for mt in range(N_MC):
                            op_pending.append(op_chunk(qc - 1, mt))
                if pend_S is None:
                    pend_S = emit_S(b)
                sp, off, diag = pend_S
                f = filler_map.get(i)
                if f is not None:
                    f()
                elif op_pending and i >= 10 and \
                        (len(op_pending) >= 6 or i % 2 == 1):
                    # spread the last chunks out so PE filler survives into
                    # the final (ACT-bound) kt sweep
                    op_pending.pop(0)()
                pend_S = emit_S(blocks[i + 1]) \
                    if i + 1 < len(blocks) else None
                pt = emit_exp(sp, off, diag)
                emit_PV(b, oacc_cur, pt, off)
                if kt == nkt - 1:
                    norm_pair(t, qc, oacc_cur, last=(i == len(blocks) - 1))
            # tail: remaining queued chunks + outproj of the last q-chunk
            while op_pending:
                op_pending.pop(0)()
            for mt in range(N_MC):
                op_chunk(N_QC - 1, mt, tail=True)()

    nc.compile()
    return nc


def make_in_maps(x, key_weight, query_weight, value_weight, output_weight,
                 sines, cosines):
    """Host-side sharding + layout prep. Returns list of 8 per-core dicts."""
    import ml_dtypes
    bf16 = ml_dtypes.bfloat16
    f32 = np.float32

    # RoPE factor tiles [128, S]: row r (within a 64-channel head block)
    # carries cos/sin of pair index (r % 64) // 2; sin rows get sign -1 on
    # even rows (out_even = e*c - o*s) and +1 on odd rows.
    idx = np.tile(np.repeat(np.arange(D_HEAD // 2), 2), 2)  # [128]
    sign = np.tile(np.array([-1.0, 1.0], dtype=f32), 64)
    cosT = np.ascontiguousarray(cosines.T[idx, :]).astype(bf16)          # [128, S]
    sinT = sines.T[idx, :] * sign[:, None]
    # rows pre-permuted by the pair swap so that P @ (x * sinswapT) equals
    # rot(x) * sinT
    rr128 = np.arange(128) ^ 1
    sinswapT = np.ascontiguousarray(sinT[rr128, :]).astype(bf16)

    psw = np.zeros((128, 128), dtype=f32)
    rr = np.arange(128)
    psw[rr, rr ^ 1] = 1.0

    # pre-exp causal masking: add -1e9 above the diagonal of the boundary
    # sub-block (via triA.T @ identB on the PE)
    triA = np.where(np.arange(128)[None, :] > np.arange(128)[:, None],
                    np.float32(-1e9), np.float32(0.0)).astype(bf16)
    identB = np.eye(128, dtype=np.float32).astype(bf16)

    in_maps = []
    for c in range(N_CORES):
        b, g = divmod(c, 4)
        hs = slice(g * HEADS_PER_CORE, (g + 1) * HEADS_PER_CORE)
        xTb = np.ascontiguousarray(x[b].T).astype(bf16)
        wqT = np.ascontiguousarray(
            query_weight[hs].transpose(2, 0, 1).reshape(D_MODEL, 256)).astype(bf16)
        wkT = np.ascontiguousarray(
            key_weight[hs].transpose(2, 0, 1).reshape(D_MODEL, 256)).astype(bf16)
        wvT = np.ascontiguousarray(
            value_weight[hs].transpose(2, 0, 1).reshape(D_MODEL, 256)).astype(bf16)
        woT = np.ascontiguousarray(
            output_weight[:, hs, :].transpose(1, 2, 0).reshape(256, D_MODEL)
        ).astype(bf16)
        in_maps.append({
            "xT": xTb,
            "wqkT": np.concatenate([wqT, wkT], axis=1),
            "wvT": wvT,
            "woT": woT,
            "cosT": cosT,
            "sinswapT": sinswapT,
            "pswap": psw,
            "triA": triA,
            "identB": identB,
        })
    return in_maps


_NC_CACHE = None


def get_nc():
    global _NC_CACHE
    if _NC_CACHE is None:
        _NC_CACHE = build_nc()
    return _NC_CACHE


def kernel(x, key_weight, query_weight, value_weight, output_weight,
           sines, cosines, _trace=False, _trace_kwargs=None):
    in_maps = make_in_maps(x, key_weight, query_weight, value_weight,
                           output_weight, sines, cosines)
    nc = get_nc()
    kw = {}
    if _trace:
        kw = dict(trace=True, **(_trace_kwargs or {}))
    res = run_bass_kernel_spmd(nc, in_maps, core_ids=list(range(N_CORES)), **kw)
    out = np.zeros((B, S, D_MODEL), dtype=np.float32)
    for c in range(N_CORES):
        b = c // 4
        out[b] += res.results[c]["outT"].astype(np.float32).T
    kernel.last_result = res
    return out
